# revision 3
# baseline (speedup 1.0000x reference)
"""Multi-head attention (B=4, S=2048, E=1024, H=16) on 8 TRN2 NeuronCores.

Sharding: core c -> (batch b = c//2, head-half hh = c%2  => 8 heads = 512 features).
All-transposed dataflow, f32r matmuls, softmax without max-subtraction
(scores are bounded |s| <~ 3 by construction), exp split between ScalarE
(spline LUT) and a custom 8-stage VectorE op, rowsums via a ones-column in
the ctx matmul's stationary operand, normalization folded into PSUM eviction.
Host does input transposes / f32r pre-rounding and the final pairwise
partial-sum + bias + transpose.
"""

import sys

sys.path.insert(0, "/opt/trn_rl_repo")

import numpy as np

B, S, E, H = 4, 2048, 1024, 16
DK = E // H  # 64
NCORES = 8
F = 512  # features per core (head-half)
SCALE = 1.0 / 8.0  # 1/sqrt(DK)

# ---------------------------------------------------------------- helpers

def _round_f32r(x: np.ndarray) -> np.ndarray:
    """Round fp32 to f32r (1s+8e+11m = top 20 bits), round-to-nearest-even."""
    u = np.ascontiguousarray(x, dtype=np.float32).view(np.uint32).astype(np.uint64)
    u = (u + 0x7FF + ((u >> 12) & 1)) & 0xFFFFF000
    return u.astype(np.uint32).view(np.float32)


_EXP_OPS = None


def _register_exp_ops():
    """Two custom DVE ops for exp(x/8) on raw scores |x| <= ~24:
    EXPA_ANT: q = (((c3*x + c2)*x + c1)*x + 1)^4  ~= exp(x/128)
    SQ4_ANT:  out = in^16  (4 squarings)  => exp(x/8).
    """
    global _EXP_OPS
    if _EXP_OPS is not None:
        return _EXP_OPS
    import concourse.dve_ops as dve_ops
    from concourse.dve_ops import DveOp, DveOpSpec, get_dve_sub_opcode
    from concourse.dve_spec import Spec, Src0, C0, C1, C2, One, sq, lower

    existing = {op.name: op for op in dve_ops.OPS}
    if "EXPA_ANT" in existing and "SQ4_ANT" in existing:
        _EXP_OPS = (existing["EXPA_ANT"], existing["SQ4_ANT"])
        return _EXP_OPS

    def _ref_a(in0, in1, c0, c1, c2):
        x = in0.astype(np.float32)
        q = ((x * np.float32(c2) + np.float32(c1)) * x + np.float32(c0)) * x + np.float32(1.0)
        q = q * q
        return q * q

    def _ref_sq4(in0, in1, c0, c1, c2):
        x = in0.astype(np.float32)
        for _ in range(4):
            x = x * x
        return x

    opa = DveOp(
        "EXPA_ANT",
        Spec(body=sq(sq(((Src0 * C2 + C1) * Src0 + C0) * Src0 + One)), reference=_ref_a),
        subdim=False,
        uops_sha={},
    )
    opb = DveOp(
        "SQ4_ANT",
        Spec(body=sq(sq(sq(sq(Src0)))), reference=_ref_sq4),
        subdim=False,
        uops_sha={},
    )
    for op in (opa, opb):
        dve_ops.OPS.append(op)
        dve_ops._SUB_OPCODE_FOR_NAME[op.name] = (
            max(dve_ops._SUB_OPCODE_FOR_NAME.values()) + 1
        )
        dve_ops.CUSTOM_DVE_SPECS[op.name] = op.spec
        for ver in ("v3", "v4"):
            try:
                spec_c = DveOpSpec(
                    name=op.name,
                    opcode=get_dve_sub_opcode(op.name),
                    uops=lower(op.spec, ver=ver),
                    rd1_en=False,
                )
                op.uops_sha[ver] = spec_c.sha(ver)
            except Exception:
                pass
    _EXP_OPS = (opa, opb)
    return _EXP_OPS


EXPA_CONSTS = {
    "s0": 1.0 / 512.0,
    "s1": 1.0 / (2.0 * 512.0**2),
    "imm2": 1.0 / (6.0 * 512.0**3),
}

_BUILT = None  # cached compiled Bass program


def _build_program():
    global _BUILT
    if _BUILT is not None:
        return _BUILT

    import concourse.bass as bass
    import concourse.mybir as mybir
    from concourse import bacc
    from concourse.tile import TileContext
    from concourse.masks import make_identity

    EXPA, SQ4 = _register_exp_ops()

    F32 = mybir.dt.float32
    F32R = mybir.dt.float32r
    BF16 = mybir.dt.bfloat16
    AF = mybir.ActivationFunctionType

    nc = bacc.Bacc("TRN2", target_bir_lowering=False, debug=False, num_devices=NCORES)

    xq = nc.dram_tensor("xq", [E, S], F32R, kind="ExternalInput")
    xk = nc.dram_tensor("xk", [E, S], F32R, kind="ExternalInput")
    xv = nc.dram_tensor("xv", [E, S], F32R, kind="ExternalInput")
    wq = nc.dram_tensor("wq", [E, F], F32R, kind="ExternalInput")
    wk = nc.dram_tensor("wk", [E, F], F32R, kind="ExternalInput")
    wv = nc.dram_tensor("wv", [E, F], F32R, kind="ExternalInput")
    wo = nc.dram_tensor("wo", [F, E], F32R, kind="ExternalInput")
    bq = nc.dram_tensor("bq", [F], F32, kind="ExternalInput")
    bk = nc.dram_tensor("bk", [F], F32, kind="ExternalInput")
    bv = nc.dram_tensor("bv", [F], F32, kind="ExternalInput")
    out_d = nc.dram_tensor("out", [E, S], F32, kind="ExternalOutput")

    with TileContext(nc) as tc:
        with (
            tc.tile_pool(name="persist", bufs=1) as persist,
            tc.tile_pool(name="xp", bufs=4) as xp,
            tc.tile_pool(name="wp", bufs=2) as wp,
            tc.tile_pool(name="vs", bufs=2) as vs,
            tc.tile_pool(name="ptp", bufs=5) as ptp,
            tc.tile_pool(name="smp", bufs=2) as smp,
            tc.tile_pool(name="ost", bufs=2) as ostp,
            tc.tile_pool(name="mm", bufs=2, space="PSUM") as mm,
            tc.tile_pool(name="scp", bufs=3, space="PSUM") as scp,
            tc.tile_pool(name="cxp", bufs=3, space="PSUM") as cxp,
        ):
            QT = persist.tile([128, 4, S], BF16)
            KT = persist.tile([128, 4, S], BF16)
            Vn = persist.tile([128, 16, 8, 65], F32R)
            CX = persist.tile([128, 4, S], F32R)

            ident = persist.tile([128, 128], F32)
            make_identity(nc, ident)
            onec = persist.tile([128, 8], F32)
            nc.vector.memset(onec, 1.0)

            biases = persist.tile([128, 3, 4], F32)
            for ti, bt in enumerate((bq, bk, bv)):
                nc.sync.dma_start(
                    out=biases[:, ti, :], in_=bt.rearrange("(ft p) -> p ft", p=128)
                )

            # ---------------- P1: projections (+ V transpose) ----------------
            for xt, wt, ti in ((xv, wv, 2), (xk, wk, 1), (xq, wq, 0)):
                wt_r = wt.rearrange("(ec p) f -> p ec f", p=128)
                wfulA = wp.tile([128, 4, F], F32R, tag="w")
                wfulB = wp.tile([128, 4, F], F32R, tag="w")
                nc.sync.dma_start(out=wfulA, in_=wt_r[:, 0:4, :])
                nc.sync.dma_start(out=wfulB, in_=wt_r[:, 4:8, :])
                xt_r = xt.rearrange("(ec p) s -> p ec s", p=128)
                for sb_ in range(4):
                    ssl = slice(sb_ * 512, (sb_ + 1) * 512)
                    xh0 = xp.tile([128, 4, 512], F32R, tag="x")
                    xh1 = xp.tile([128, 4, 512], F32R, tag="x")
                    nc.sync.dma_start(out=xh0, in_=xt_r[:, 0:4, ssl])
                    nc.sync.dma_start(out=xh1, in_=xt_r[:, 4:8, ssl])
                    for ft in range(4):
                        fsl = slice(ft * 128, (ft + 1) * 128)
                        p = mm.tile([128, 512], F32, tag="mm")
                        for ec in range(8):
                            xsrc = xh0 if ec < 4 else xh1
                            wsrc = wfulA if ec < 4 else wfulB
                            nc.tensor.matmul(
                                p,
                                wsrc[:, ec % 4, fsl],
                                xsrc[:, ec % 4, :],
                                start=(ec == 0),
                                stop=(ec == 7),
                            )
                        if ti != 2:
                            dst = QT if ti == 0 else KT
                            nc.vector.tensor_scalar_add(
                                out=dst[:, ft, ssl],
                                in0=p,
                                scalar1=biases[:, ti, ft : ft + 1],
                            )
                        else:
                            vt = vs.tile([128, 512], F32, tag="vs")
                            nc.vector.tensor_scalar_add(
                                out=vt, in0=p, scalar1=biases[:, ti, ft : ft + 1]
                            )
                            for st in range(4):
                                tp = mm.tile([128, 512], F32, tag="mm")
                                nc.tensor.transpose(
                                    tp[:, 0:128], vt[:, st * 128 : (st + 1) * 128], ident
                                )
                                kti = sb_ * 4 + st
                                nc.vector.tensor_copy(
                                    out=Vn[:, kti, 2 * ft : 2 * ft + 2, 0:64],
                                    in_=tp[:, 0:128].rearrange(
                                        "p (h d) -> p h d", h=2
                                    ),
                                )
            for kti in range(16):
                nc.vector.tensor_copy(
                    out=Vn[:, kti, :, 64:65].rearrange("p h one -> p (h one)"),
                    in_=onec,
                )

            # ---------------- P2: attention + fused output projection ------
            wo_r = wo.rearrange("(fc p) e -> p fc e", p=128)
            wofA = wp.tile([128, 2, E], F32R, tag="w")
            wofB = wp.tile([128, 2, E], F32R, tag="w")
            nc.sync.dma_start(out=wofA, in_=wo_r[:, 0:2, :])
            nc.sync.dma_start(out=wofB, in_=wo_r[:, 2:4, :])
            for qb in range(4):
                qsl = slice(qb * 512, (qb + 1) * 512)
                for pr in range(4):
                    c0 = cxp.tile([65, 512], F32, tag="cx")
                    c1 = cxp.tile([65, 512], F32, tag="cx")
                    for kt in range(16):
                        ksl = slice(kt * 128, (kt + 1) * 128)
                        s0 = scp.tile([128, 512], F32, tag="sc")
                        s1 = scp.tile([128, 512], F32, tag="sc")
                        nc.tensor.matmul(
                            s0, KT[0:64, pr, ksl], QT[0:64, pr, qsl],
                            start=True, stop=True, tile_position=(0, 0),
                        )
                        nc.tensor.matmul(
                            s1, KT[64:128, pr, ksl], QT[64:128, pr, qsl],
                            start=True, stop=True, tile_position=(64, 0),
                        )
                        pt0 = ptp.tile([128, 512], F32R, tag="pt")
                        pt1 = ptp.tile([128, 512], F32R, tag="pt")
                        # split exp work between ScalarE and VectorE (2-pass)
                        nc.scalar.activation(out=pt0, in_=s0, func=AF.Exp, scale=SCALE)
                        escr = ptp.tile([128, 512], F32, tag="escr")
                        nc.vector._custom_dve(EXPA, out=escr, in0=s1, **EXPA_CONSTS)
                        nc.vector._custom_dve(SQ4, out=pt1, in0=escr)
                        nc.tensor.matmul(
                            c0, Vn[:, kt, 2 * pr, :], pt0,
                            start=(kt == 0), stop=(kt == 15),
                        )
                        nc.tensor.matmul(
                            c1, Vn[:, kt, 2 * pr + 1, :], pt1,
                            start=(kt == 0), stop=(kt == 15),
                        )
                    # normalize + evict:  CX[:, pr, qsl] = ctx / rowsum
                    sums0 = smp.tile([1, 512], F32, tag="sums")
                    sums1 = smp.tile([1, 512], F32, tag="sums")
                    nc.vector.tensor_copy(out=sums0, in_=c0[64:65, :])
                    nc.vector.tensor_copy(out=sums1, in_=c1[64:65, :])
                    inv0 = smp.tile([1, 512], F32, tag="inv")
                    inv1 = smp.tile([1, 512], F32, tag="inv")
                    scr0 = smp.tile([1, 512], F32, tag="scr")
                    scr1 = smp.tile([1, 512], F32, tag="scr")
                    nc.vector.reciprocal_approx_accurate(
                        out=inv0, in_=sums0, scratch=scr0
                    )
                    nc.vector.reciprocal_approx_accurate(
                        out=inv1, in_=sums1, scratch=scr1
                    )
                    invB0 = smp.tile([64, 512], F32, tag="invB")
                    invB1 = smp.tile([64, 512], F32, tag="invB")
                    nc.gpsimd.partition_broadcast(out_ap=invB0, in_ap=inv0[0:1, :])
                    nc.gpsimd.partition_broadcast(out_ap=invB1, in_ap=inv1[0:1, :])
                    nc.vector.tensor_mul(CX[0:64, pr, qsl], c0[0:64, :], invB0)
                    nc.vector.tensor_mul(CX[64:128, pr, qsl], c1[0:64, :], invB1)
                # fused output projection for this q block
                for et in range(8):
                    esl = slice(et * 128, (et + 1) * 128)
                    p = mm.tile([128, 512], F32, tag="mm")
                    for fc in range(4):
                        wsrc = wofA if fc < 2 else wofB
                        nc.tensor.matmul(
                            p, wsrc[:, fc % 2, esl], CX[:, fc, qsl],
                            start=(fc == 0), stop=(fc == 3),
                        )
                    o = ostp.tile([128, 512], F32, tag="ost")
                    nc.vector.tensor_copy(out=o, in_=p)
                    nc.sync.dma_start(out=out_d[esl, qsl], in_=o)

    nc.compile()
    _BUILT = nc
    return nc


def _make_in_maps(inputs) -> list:
    query = np.asarray(inputs["query"], dtype=np.float32)
    key_ = np.asarray(inputs["key_"], dtype=np.float32)
    value = np.asarray(inputs["value"], dtype=np.float32)
    Wq = np.asarray(inputs["Wq"], dtype=np.float32)
    bq = np.asarray(inputs["bq"], dtype=np.float32)
    Wk = np.asarray(inputs["Wk"], dtype=np.float32)
    bk = np.asarray(inputs["bk"], dtype=np.float32)
    Wv = np.asarray(inputs["Wv"], dtype=np.float32)
    bv = np.asarray(inputs["bv"], dtype=np.float32)
    Wo = np.asarray(inputs["Wo"], dtype=np.float32)

    WqT = _round_f32r(Wq.T)  # [E_in, E_out]
    WkT = _round_f32r(Wk.T)
    WvT = _round_f32r(Wv.T)
    WoT = _round_f32r(Wo.T)  # [F_in, E_out]

    in_maps = []
    for c in range(NCORES):
        b = c // 2
        hh = c % 2
        fsl = slice(hh * F, (hh + 1) * F)
        in_maps.append(
            {
                "xq": _round_f32r(query[b].T),
                "xk": _round_f32r(key_[b].T),
                "xv": _round_f32r(value[b].T),
                "wq": np.ascontiguousarray(WqT[:, fsl]),
                "wk": np.ascontiguousarray(WkT[:, fsl]),
                "wv": np.ascontiguousarray(WvT[:, fsl]),
                "wo": np.ascontiguousarray(WoT[fsl, :]),
                "bq": np.ascontiguousarray(bq[fsl]),
                "bk": np.ascontiguousarray(bk[fsl]),
                "bv": np.ascontiguousarray(bv[fsl]),
            }
        )
    return in_maps


def kernel(**inputs) -> np.ndarray:
    from concourse.bass_utils import run_bass_kernel_spmd

    bo = np.asarray(inputs["bo"], dtype=np.float32)
    nc = _build_program()
    in_maps = _make_in_maps(inputs)

    res = run_bass_kernel_spmd(nc, in_maps, core_ids=list(range(NCORES)))

    out = np.empty((B, S, E), dtype=np.float32)
    for b in range(B):
        partial = res.results[2 * b]["out"] + res.results[2 * b + 1]["out"]  # [E, S]
        out[b] = partial.T + bo[None, :]
    return out



# revision 8
# speedup vs baseline: 1.5986x; 1.5986x over previous
"""Multi-head attention (B=4, S=2048, E=1024, H=16) on 8 TRN2 NeuronCores.

Sharding: core c -> (batch b = c//2, head-half hh = c%2  => 8 heads = 512 features).
v2: all-bf16 matmuls (HW runs bf16 ~1.6x faster than f32r), V^T computed
directly (x stationary, w moving) so no PE transposes, single-pass 8-stage
DVE exp (cubic*sq*sq = p(x)^4 ~= exp(x/8)) paired with ScalarE spline exp,
evictions spread across ScalarE (bias-add) and GpSimd (V bias, softmax
normalization). Rowsums via a ones-column in the ctx stationary operand.
Host does input transposes / bf16 rounding and the final pairwise
partial-sum + bias + transpose.
"""

import sys

sys.path.insert(0, "/opt/trn_rl_repo")

import numpy as np

B, S, E, H = 4, 2048, 1024, 16
DK = E // H  # 64
NCORES = 8
F = 512  # features per core (head-half)
SCALE = 1.0 / 8.0  # 1/sqrt(DK)

# cubic p(y) ~= e^y on |y| <= 0.85 (relative-minimax IRLS fit); the DVE op
# computes p(x/32)^4 ~= exp(x/8) for raw scores |x| <= 27.
_A1 = 1.0065252287600177
_A2 = 0.5243464329472075
_A3 = 0.15847803780651157
EXP_CONSTS = {
    "s0": _A1 / 32.0,
    "s1": _A2 / (32.0 * 32.0),
    "imm2": _A3 / (32.0 * 32.0 * 32.0),
}

# ---------------------------------------------------------------- helpers

_EXP_OPS = None


def _register_exp_ops():
    """Custom DVE op for exp via  q = (((c2*x + c1)*x + c0)*x + 1)^4."""
    global _EXP_OPS
    if _EXP_OPS is not None:
        return _EXP_OPS
    import concourse.dve_ops as dve_ops
    from concourse.dve_ops import DveOp, DveOpSpec, get_dve_sub_opcode
    from concourse.dve_spec import Spec, Src0, C0, C1, C2, One, sq, lower

    existing = {op.name: op for op in dve_ops.OPS}
    if "EXPA_ANT" in existing:
        _EXP_OPS = existing["EXPA_ANT"]
        return _EXP_OPS

    def _ref_a(in0, in1, c0, c1, c2):
        x = in0.astype(np.float32)
        q = ((x * np.float32(c2) + np.float32(c1)) * x + np.float32(c0)) * x + np.float32(1.0)
        q = q * q
        return q * q

    opa = DveOp(
        "EXPA_ANT",
        Spec(body=sq(sq(((Src0 * C2 + C1) * Src0 + C0) * Src0 + One)), reference=_ref_a),
        subdim=False,
        uops_sha={},
    )
    dve_ops.OPS.append(opa)
    dve_ops._SUB_OPCODE_FOR_NAME[opa.name] = (
        max(dve_ops._SUB_OPCODE_FOR_NAME.values()) + 1
    )
    dve_ops.CUSTOM_DVE_SPECS[opa.name] = opa.spec
    for ver in ("v3", "v4"):
        try:
            spec_c = DveOpSpec(
                name=opa.name,
                opcode=get_dve_sub_opcode(opa.name),
                uops=lower(opa.spec, ver=ver),
                rd1_en=False,
            )
            opa.uops_sha[ver] = spec_c.sha(ver)
        except Exception:
            pass
    _EXP_OPS = opa
    return opa


_BUILT = None  # cached compiled Bass program


def _build_program():
    global _BUILT
    if _BUILT is not None:
        return _BUILT

    import concourse.bass as bass
    import concourse.mybir as mybir
    from concourse import bacc
    from concourse.tile import TileContext

    EXPA = _register_exp_ops()

    F32 = mybir.dt.float32
    BF16 = mybir.dt.bfloat16
    AF = mybir.ActivationFunctionType

    nc = bacc.Bacc("TRN2", target_bir_lowering=False, debug=False, num_devices=NCORES)

    xq = nc.dram_tensor("xq", [E, S], BF16, kind="ExternalInput")
    xk = nc.dram_tensor("xk", [E, S], BF16, kind="ExternalInput")
    xv = nc.dram_tensor("xv", [E, S], BF16, kind="ExternalInput")
    wq = nc.dram_tensor("wq", [E, F], BF16, kind="ExternalInput")
    wk = nc.dram_tensor("wk", [E, F], BF16, kind="ExternalInput")
    wv = nc.dram_tensor("wv", [E, F], BF16, kind="ExternalInput")
    wo = nc.dram_tensor("wo", [F, E], BF16, kind="ExternalInput")
    bq = nc.dram_tensor("bq", [F], F32, kind="ExternalInput")
    bk = nc.dram_tensor("bk", [F], F32, kind="ExternalInput")
    bv = nc.dram_tensor("bv", [F], F32, kind="ExternalInput")
    out_d = nc.dram_tensor("out", [E, S], F32, kind="ExternalOutput")

    with TileContext(nc) as tc:
        with (
            tc.tile_pool(name="persist", bufs=1) as persist,
            tc.tile_pool(name="xp", bufs=4) as xp,
            tc.tile_pool(name="wp", bufs=2) as wp,
            tc.tile_pool(name="ptp", bufs=5) as ptp,
            tc.tile_pool(name="smp", bufs=2) as smp,
            tc.tile_pool(name="ost", bufs=2) as ostp,
            tc.tile_pool(name="ps", bufs=5, space="PSUM") as ps,
            tc.tile_pool(name="cxp", bufs=3, space="PSUM") as cxp,
        ):
            QT = persist.tile([128, 4, S], BF16)
            KT = persist.tile([128, 4, S], BF16)
            Vn = persist.tile([128, 16, 8, 65], BF16)
            CX = persist.tile([128, 4, S], BF16)

            # ones column of Vn (rowsum trick)
            nc.vector.memset(Vn[:, :, :, 64:65], 1.0)

            # per-partition biases for Q,K evictions
            biases = persist.tile([128, 2, 4], F32)
            for ti, bt in enumerate((bq, bk)):
                nc.sync.dma_start(
                    out=biases[:, ti, :], in_=bt.rearrange("(ft p) -> p ft", p=128)
                )
            # bv broadcast along partitions (V^T bias is along the free dim)
            bvrow = persist.tile([1, F], F32)
            nc.sync.dma_start(out=bvrow, in_=bv.rearrange("(one f) -> one f", one=1))
            bvtile = persist.tile([128, F], F32)
            nc.gpsimd.partition_broadcast(out_ap=bvtile, in_ap=bvrow)

            # ---------------- P1: projections ----------------
            # V^T computed directly: stationary x chunk, moving wv.
            for xt, wt, ti in ((xv, wv, 2), (xk, wk, 1), (xq, wq, 0)):
                wt_r = wt.rearrange("(ec p) f -> p ec f", p=128)
                wful = wp.tile([128, 8, F], BF16, tag="w")
                nc.sync.dma_start(out=wful[:, 0:4, :], in_=wt_r[:, 0:4, :])
                nc.sync.dma_start(out=wful[:, 4:8, :], in_=wt_r[:, 4:8, :])
                xt_r = xt.rearrange("(ec p) s -> p ec s", p=128)
                for sb_ in range(4):
                    ssl = slice(sb_ * 512, (sb_ + 1) * 512)
                    xh = xp.tile([128, 8, 512], BF16, tag="x")
                    nc.sync.dma_start(out=xh[:, 0:4, :], in_=xt_r[:, 0:4, ssl])
                    nc.sync.dma_start(out=xh[:, 4:8, :], in_=xt_r[:, 4:8, ssl])
                    if ti == 2:
                        # V^T: out block [s 128, f 512] per kt
                        for st in range(4):
                            kt = sb_ * 4 + st
                            stsl = slice(st * 128, (st + 1) * 128)
                            p = ps.tile([128, 512], F32, tag="sc")
                            for ec in range(8):
                                nc.tensor.matmul(
                                    p,
                                    xh[:, ec, stsl],
                                    wful[:, ec, :],
                                    start=(ec == 0),
                                    stop=(ec == 7),
                                )
                            nc.vector.tensor_tensor(
                                out=Vn[:, kt, :, 0:64],
                                in0=p.rearrange("p (h d) -> p h d", h=8),
                                in1=bvtile.rearrange("p (h d) -> p h d", h=8),
                                op=mybir.AluOpType.add,
                            )
                    else:
                        # Q/K: stationary w block, moving x
                        dst = QT if ti == 0 else KT
                        for ft in range(4):
                            fsl = slice(ft * 128, (ft + 1) * 128)
                            p = ps.tile([128, 512], F32, tag="sc")
                            for ec in range(8):
                                nc.tensor.matmul(
                                    p,
                                    wful[:, ec, fsl],
                                    xh[:, ec, :],
                                    start=(ec == 0),
                                    stop=(ec == 7),
                                )
                            nc.scalar.activation(
                                out=dst[:, ft, ssl],
                                in_=p,
                                func=AF.Identity,
                                bias=biases[:, ti, ft : ft + 1],
                            )

            # ---------------- P2: attention + fused output projection ------
            wo_r = wo.rearrange("(fc p) e -> p fc e", p=128)
            wof = wp.tile([128, 4, E], BF16, tag="w")
            nc.sync.dma_start(out=wof[:, 0:2, :], in_=wo_r[:, 0:2, :])
            nc.sync.dma_start(out=wof[:, 2:4, :], in_=wo_r[:, 2:4, :])
            for qb in range(4):
                qsl = slice(qb * 512, (qb + 1) * 512)
                for pr in range(4):
                    c0 = cxp.tile([65, 512], F32, tag="cx")
                    c1 = cxp.tile([65, 512], F32, tag="cx")

                    s_tiles = {}

                    def emit_scores(kt, pr=pr, qsl=qsl, s_tiles=s_tiles):
                        ksl = slice(kt * 128, (kt + 1) * 128)
                        s0 = ps.tile([128, 512], F32, tag="sc")
                        s1 = ps.tile([128, 512], F32, tag="sc")
                        nc.tensor.matmul(
                            s0, KT[0:64, pr, ksl], QT[0:64, pr, qsl],
                            start=True, stop=True, tile_position=(0, 0),
                        )
                        nc.tensor.matmul(
                            s1, KT[64:128, pr, ksl], QT[64:128, pr, qsl],
                            start=True, stop=True, tile_position=(64, 0),
                        )
                        s_tiles[kt] = (s0, s1)

                    emit_scores(0)
                    for kt in range(16):
                        if kt + 1 < 16:
                            emit_scores(kt + 1)
                        s0, s1 = s_tiles.pop(kt)
                        pt0 = ptp.tile([128, 512], BF16, tag="pt")
                        pt1 = ptp.tile([128, 512], BF16, tag="pt")
                        # exp split between ScalarE (spline LUT) and VectorE
                        nc.scalar.activation(out=pt0, in_=s0, func=AF.Exp, scale=SCALE)
                        if kt == 5:
                            nc.scalar.activation(
                                out=pt1, in_=s1, func=AF.Exp, scale=SCALE
                            )
                        else:
                            nc.vector._custom_dve(EXPA, out=pt1, in0=s1, **EXP_CONSTS)
                        nc.tensor.matmul(
                            c0, Vn[:, kt, 2 * pr, :], pt0,
                            start=(kt == 0), stop=(kt == 15),
                        )
                        nc.tensor.matmul(
                            c1, Vn[:, kt, 2 * pr + 1, :], pt1,
                            start=(kt == 0), stop=(kt == 15),
                        )
                    # normalize + evict:  CX[:, pr, qsl] = ctx / rowsum
                    sums0 = smp.tile([1, 512], F32, tag="sums")
                    sums1 = smp.tile([1, 512], F32, tag="sums")
                    nc.scalar.activation(out=sums0, in_=c0[64:65, :], func=AF.Copy)
                    nc.scalar.activation(out=sums1, in_=c1[64:65, :], func=AF.Copy)
                    inv0 = smp.tile([1, 512], F32, tag="inv")
                    inv1 = smp.tile([1, 512], F32, tag="inv")
                    nc.vector.reciprocal_approx_fast(out=inv0, in_=sums0)
                    nc.vector.reciprocal_approx_fast(out=inv1, in_=sums1)
                    invB0 = smp.tile([64, 512], F32, tag="invB")
                    invB1 = smp.tile([64, 512], F32, tag="invB")
                    nc.gpsimd.partition_broadcast(out_ap=invB0, in_ap=inv0[0:1, :])
                    nc.gpsimd.partition_broadcast(out_ap=invB1, in_ap=inv1[0:1, :])
                    nc.vector.tensor_tensor(
                        out=CX[0:64, pr, qsl], in0=c0[0:64, :], in1=invB0,
                        op=mybir.AluOpType.mult,
                    )
                    nc.vector.tensor_tensor(
                        out=CX[64:128, pr, qsl], in0=c1[0:64, :], in1=invB1,
                        op=mybir.AluOpType.mult,
                    )
                # fused output projection for this q block
                for et in range(8):
                    esl = slice(et * 128, (et + 1) * 128)
                    p = ps.tile([128, 512], F32, tag="sc")
                    for fc in range(4):
                        nc.tensor.matmul(
                            p, wof[:, fc, esl], CX[:, fc, qsl],
                            start=(fc == 0), stop=(fc == 3),
                        )
                    o = ostp.tile([128, 512], F32, tag="ost")
                    nc.scalar.activation(out=o, in_=p, func=AF.Copy)
                    nc.sync.dma_start(out=out_d[esl, qsl], in_=o)

    nc.compile()
    _BUILT = nc
    return nc


def _make_in_maps(inputs) -> list:
    import ml_dtypes

    bf16 = ml_dtypes.bfloat16
    query = np.asarray(inputs["query"], dtype=np.float32)
    key_ = np.asarray(inputs["key_"], dtype=np.float32)
    value = np.asarray(inputs["value"], dtype=np.float32)
    Wq = np.asarray(inputs["Wq"], dtype=np.float32)
    bq = np.asarray(inputs["bq"], dtype=np.float32)
    Wk = np.asarray(inputs["Wk"], dtype=np.float32)
    bk = np.asarray(inputs["bk"], dtype=np.float32)
    Wv = np.asarray(inputs["Wv"], dtype=np.float32)
    bv = np.asarray(inputs["bv"], dtype=np.float32)
    Wo = np.asarray(inputs["Wo"], dtype=np.float32)

    WqT = Wq.T.astype(bf16)  # [E_in, E_out]
    WkT = Wk.T.astype(bf16)
    WvT = Wv.T.astype(bf16)
    WoT = Wo.T.astype(bf16)  # [F_in, E_out]

    xqs = [np.ascontiguousarray(query[b].T).astype(bf16) for b in range(B)]
    xks = [np.ascontiguousarray(key_[b].T).astype(bf16) for b in range(B)]
    xvs = [np.ascontiguousarray(value[b].T).astype(bf16) for b in range(B)]

    in_maps = []
    for c in range(NCORES):
        b = c // 2
        hh = c % 2
        fsl = slice(hh * F, (hh + 1) * F)
        in_maps.append(
            {
                "xq": xqs[b],
                "xk": xks[b],
                "xv": xvs[b],
                "wq": np.ascontiguousarray(WqT[:, fsl]),
                "wk": np.ascontiguousarray(WkT[:, fsl]),
                "wv": np.ascontiguousarray(WvT[:, fsl]),
                "wo": np.ascontiguousarray(WoT[fsl, :]),
                "bq": np.ascontiguousarray(bq[fsl]),
                "bk": np.ascontiguousarray(bk[fsl]),
                "bv": np.ascontiguousarray(bv[fsl]),
            }
        )
    return in_maps


def kernel(**inputs) -> np.ndarray:
    from concourse.bass_utils import run_bass_kernel_spmd

    bo = np.asarray(inputs["bo"], dtype=np.float32)
    nc = _build_program()
    in_maps = _make_in_maps(inputs)

    res = run_bass_kernel_spmd(nc, in_maps, core_ids=list(range(NCORES)))

    out = np.empty((B, S, E), dtype=np.float32)
    for b in range(B):
        partial = res.results[2 * b]["out"] + res.results[2 * b + 1]["out"]  # [E, S]
        out[b] = partial.T + bo[None, :]
    return out


# revision 9
# speedup vs baseline: 1.6964x; 1.0611x over previous
"""Multi-head attention (B=4, S=2048, E=1024, H=16) on 8 TRN2 NeuronCores.

Sharding: core c -> (batch b = c//2, head-half hh = c%2  => 8 heads = 512 features).
v2: all-bf16 matmuls (HW runs bf16 ~1.6x faster than f32r), V^T computed
directly (x stationary, w moving) so no PE transposes, single-pass 8-stage
DVE exp (cubic*sq*sq = p(x)^4 ~= exp(x/8)) paired with ScalarE spline exp,
evictions spread across ScalarE (bias-add) and GpSimd (V bias, softmax
normalization). Rowsums via a ones-column in the ctx stationary operand.
Host does input transposes / bf16 rounding and the final pairwise
partial-sum + bias + transpose.
"""

import sys

sys.path.insert(0, "/opt/trn_rl_repo")

import numpy as np

B, S, E, H = 4, 2048, 1024, 16
DK = E // H  # 64
NCORES = 8
F = 512  # features per core (head-half)
SCALE = 1.0 / 8.0  # 1/sqrt(DK)

# cubic p(y) ~= e^y on |y| <= 0.85 (relative-minimax IRLS fit); the DVE op
# computes p(x/32)^4 ~= exp(x/8) for raw scores |x| <= 27.
_A1 = 1.0065252287600177
_A2 = 0.5243464329472075
_A3 = 0.15847803780651157
EXP_CONSTS = {
    "s0": _A1 / 32.0,
    "s1": _A2 / (32.0 * 32.0),
    "imm2": _A3 / (32.0 * 32.0 * 32.0),
}

# ---------------------------------------------------------------- helpers

_EXP_OPS = None


def _register_exp_ops():
    """Custom DVE op for exp via  q = (((c2*x + c1)*x + c0)*x + 1)^4."""
    global _EXP_OPS
    if _EXP_OPS is not None:
        return _EXP_OPS
    import concourse.dve_ops as dve_ops
    from concourse.dve_ops import DveOp, DveOpSpec, get_dve_sub_opcode
    from concourse.dve_spec import Spec, Src0, C0, C1, C2, One, sq, lower

    existing = {op.name: op for op in dve_ops.OPS}
    if "EXPA_ANT" in existing:
        _EXP_OPS = existing["EXPA_ANT"]
        return _EXP_OPS

    def _ref_a(in0, in1, c0, c1, c2):
        x = in0.astype(np.float32)
        q = ((x * np.float32(c2) + np.float32(c1)) * x + np.float32(c0)) * x + np.float32(1.0)
        q = q * q
        return q * q

    opa = DveOp(
        "EXPA_ANT",
        Spec(body=sq(sq(((Src0 * C2 + C1) * Src0 + C0) * Src0 + One)), reference=_ref_a),
        subdim=False,
        uops_sha={},
    )
    dve_ops.OPS.append(opa)
    dve_ops._SUB_OPCODE_FOR_NAME[opa.name] = (
        max(dve_ops._SUB_OPCODE_FOR_NAME.values()) + 1
    )
    dve_ops.CUSTOM_DVE_SPECS[opa.name] = opa.spec
    for ver in ("v3", "v4"):
        try:
            spec_c = DveOpSpec(
                name=opa.name,
                opcode=get_dve_sub_opcode(opa.name),
                uops=lower(opa.spec, ver=ver),
                rd1_en=False,
            )
            opa.uops_sha[ver] = spec_c.sha(ver)
        except Exception:
            pass
    _EXP_OPS = opa
    return opa


_BUILT = None  # cached compiled Bass program


def _build_program():
    global _BUILT
    if _BUILT is not None:
        return _BUILT

    import concourse.bass as bass
    import concourse.mybir as mybir
    from concourse import bacc
    from concourse.tile import TileContext

    EXPA = _register_exp_ops()

    F32 = mybir.dt.float32
    BF16 = mybir.dt.bfloat16
    AF = mybir.ActivationFunctionType

    nc = bacc.Bacc("TRN2", target_bir_lowering=False, debug=False, num_devices=NCORES)

    xq = nc.dram_tensor("xq", [E, S], BF16, kind="ExternalInput")
    xk = nc.dram_tensor("xk", [E, S], BF16, kind="ExternalInput")
    xv = nc.dram_tensor("xv", [E, S], BF16, kind="ExternalInput")
    wq = nc.dram_tensor("wq", [E, F], BF16, kind="ExternalInput")
    wk = nc.dram_tensor("wk", [E, F], BF16, kind="ExternalInput")
    wv = nc.dram_tensor("wv", [E, F], BF16, kind="ExternalInput")
    wo = nc.dram_tensor("wo", [F, E], BF16, kind="ExternalInput")
    bq = nc.dram_tensor("bq", [F], F32, kind="ExternalInput")
    bk = nc.dram_tensor("bk", [F], F32, kind="ExternalInput")
    bv = nc.dram_tensor("bv", [F], F32, kind="ExternalInput")
    out_d = nc.dram_tensor("out", [E, S], F32, kind="ExternalOutput")

    with TileContext(nc) as tc:
        with (
            tc.tile_pool(name="persist", bufs=1) as persist,
            tc.tile_pool(name="xp", bufs=4) as xp,
            tc.tile_pool(name="wp", bufs=2) as wp,
            tc.tile_pool(name="ptp", bufs=8) as ptp,
            tc.tile_pool(name="smp", bufs=4) as smp,
            tc.tile_pool(name="ost", bufs=2) as ostp,
            tc.tile_pool(name="ps", bufs=5, space="PSUM") as ps,
            tc.tile_pool(name="cxp", bufs=3, space="PSUM") as cxp,
        ):
            QT = persist.tile([128, 4, S], BF16)
            KT = persist.tile([128, 4, S], BF16)
            Vn = persist.tile([128, 16, 8, 65], BF16)
            CX = persist.tile([128, 4, S], BF16)

            # ones column of Vn (rowsum trick)
            nc.vector.memset(Vn[:, :, :, 64:65], 1.0)

            # per-partition biases for Q,K evictions
            biases = persist.tile([128, 2, 4], F32)
            for ti, bt in enumerate((bq, bk)):
                nc.sync.dma_start(
                    out=biases[:, ti, :], in_=bt.rearrange("(ft p) -> p ft", p=128)
                )
            # bv broadcast along partitions (V^T bias is along the free dim)
            bvrow = persist.tile([1, F], F32)
            nc.sync.dma_start(out=bvrow, in_=bv.rearrange("(one f) -> one f", one=1))
            bvtile = persist.tile([128, F], F32)
            nc.gpsimd.partition_broadcast(out_ap=bvtile, in_ap=bvrow)

            # ---------------- P1: projections ----------------
            # V^T computed directly: stationary x chunk, moving wv.
            for xt, wt, ti in ((xv, wv, 2), (xk, wk, 1), (xq, wq, 0)):
                wt_r = wt.rearrange("(ec p) f -> p ec f", p=128)
                wful = wp.tile([128, 8, F], BF16, tag="w")
                for dq in range(4):
                    nc.sync.dma_start(
                        out=wful[:, 2 * dq : 2 * dq + 2, :],
                        in_=wt_r[:, 2 * dq : 2 * dq + 2, :],
                    )
                xt_r = xt.rearrange("(ec p) s -> p ec s", p=128)
                for sb_ in range(4):
                    ssl = slice(sb_ * 512, (sb_ + 1) * 512)
                    xh = xp.tile([128, 8, 512], BF16, tag="x")
                    for dq in range(4):
                        nc.sync.dma_start(
                            out=xh[:, 2 * dq : 2 * dq + 2, :],
                            in_=xt_r[:, 2 * dq : 2 * dq + 2, ssl],
                        )
                    if ti == 2:
                        # V^T: out block [s 128, f 512] per kt
                        for st in range(4):
                            kt = sb_ * 4 + st
                            stsl = slice(st * 128, (st + 1) * 128)
                            p = ps.tile([128, 512], F32, tag="sc")
                            for ec in range(8):
                                nc.tensor.matmul(
                                    p,
                                    xh[:, ec, stsl],
                                    wful[:, ec, :],
                                    start=(ec == 0),
                                    stop=(ec == 7),
                                )
                            nc.vector.tensor_tensor(
                                out=Vn[:, kt, :, 0:64],
                                in0=p.rearrange("p (h d) -> p h d", h=8),
                                in1=bvtile.rearrange("p (h d) -> p h d", h=8),
                                op=mybir.AluOpType.add,
                            )
                    else:
                        # Q/K: stationary w block, moving x
                        dst = QT if ti == 0 else KT
                        for ft in range(4):
                            fsl = slice(ft * 128, (ft + 1) * 128)
                            p = ps.tile([128, 512], F32, tag="sc")
                            for ec in range(8):
                                nc.tensor.matmul(
                                    p,
                                    wful[:, ec, fsl],
                                    xh[:, ec, :],
                                    start=(ec == 0),
                                    stop=(ec == 7),
                                )
                            nc.scalar.activation(
                                out=dst[:, ft, ssl],
                                in_=p,
                                func=AF.Identity,
                                bias=biases[:, ti, ft : ft + 1],
                            )

            # ---------------- P2: attention + fused output projection ------
            wo_r = wo.rearrange("(fc p) e -> p fc e", p=128)
            wof = wp.tile([128, 4, E], BF16, tag="w")
            nc.sync.dma_start(out=wof[:, 0:2, :], in_=wo_r[:, 0:2, :])
            nc.sync.dma_start(out=wof[:, 2:4, :], in_=wo_r[:, 2:4, :])
            def emit_outproj(qb):
                qsl = slice(qb * 512, (qb + 1) * 512)
                def emit_et(et, qsl=qsl):
                    esl = slice(et * 128, (et + 1) * 128)
                    p = ps.tile([128, 512], F32, tag="sc")
                    for fc in range(4):
                        nc.tensor.matmul(
                            p, wof[:, fc, esl], CX[:, fc, qsl],
                            start=(fc == 0), stop=(fc == 3),
                        )
                    o = ostp.tile([128, 512], F32, tag="ost")
                    nc.scalar.activation(out=o, in_=p, func=AF.Copy)
                    nc.sync.dma_start(out=out_d[esl, qsl], in_=o)
                return emit_et

            pending = None  # (emit_et fn, next et index) for qb-1
            for qb in range(4):
                qsl = slice(qb * 512, (qb + 1) * 512)
                for pr in range(4):
                    c0 = cxp.tile([65, 512], F32, tag="cx")
                    c1 = cxp.tile([65, 512], F32, tag="cx")

                    s_tiles = {}

                    def emit_scores(kt, pr=pr, qsl=qsl, s_tiles=s_tiles):
                        ksl = slice(kt * 128, (kt + 1) * 128)
                        s0 = ps.tile([128, 512], F32, tag="sc")
                        s1 = ps.tile([128, 512], F32, tag="sc")
                        nc.tensor.matmul(
                            s0, KT[0:64, pr, ksl], QT[0:64, pr, qsl],
                            start=True, stop=True, tile_position=(0, 0),
                        )
                        nc.tensor.matmul(
                            s1, KT[64:128, pr, ksl], QT[64:128, pr, qsl],
                            start=True, stop=True, tile_position=(64, 0),
                        )
                        s_tiles[kt] = (s0, s1)

                    emit_scores(0)
                    for kt in range(16):
                        if kt + 1 < 16:
                            emit_scores(kt + 1)
                        s0, s1 = s_tiles.pop(kt)
                        pt0 = ptp.tile([128, 512], BF16, tag="pt")
                        pt1 = ptp.tile([128, 512], BF16, tag="pt")
                        # exp split between ScalarE (spline LUT) and VectorE
                        nc.scalar.activation(out=pt0, in_=s0, func=AF.Exp, scale=SCALE)
                        if kt == 5:
                            nc.scalar.activation(
                                out=pt1, in_=s1, func=AF.Exp, scale=SCALE
                            )
                        else:
                            nc.vector._custom_dve(EXPA, out=pt1, in0=s1, **EXP_CONSTS)
                        nc.tensor.matmul(
                            c0, Vn[:, kt, 2 * pr, :], pt0,
                            start=(kt == 0), stop=(kt == 15),
                        )
                        nc.tensor.matmul(
                            c1, Vn[:, kt, 2 * pr + 1, :], pt1,
                            start=(kt == 0), stop=(kt == 15),
                        )
                        # interleave previous q-block's output projection
                        if pending is not None and pr < 2 and kt % 4 == 1:
                            fn, et = pending
                            fn(et)
                            pending = (fn, et + 1) if et + 1 < 8 else None
                    # normalize + evict:  CX[:, pr, qsl] = ctx / rowsum
                    sums0 = smp.tile([1, 512], F32, tag="sums")
                    sums1 = smp.tile([1, 512], F32, tag="sums")
                    nc.scalar.activation(out=sums0, in_=c0[64:65, :], func=AF.Copy)
                    nc.scalar.activation(out=sums1, in_=c1[64:65, :], func=AF.Copy)
                    inv0 = smp.tile([1, 512], F32, tag="inv")
                    inv1 = smp.tile([1, 512], F32, tag="inv")
                    nc.vector.reciprocal_approx_fast(out=inv0, in_=sums0)
                    nc.vector.reciprocal_approx_fast(out=inv1, in_=sums1)
                    invB0 = smp.tile([64, 512], F32, tag="invB")
                    invB1 = smp.tile([64, 512], F32, tag="invB")
                    nc.gpsimd.partition_broadcast(out_ap=invB0, in_ap=inv0[0:1, :])
                    nc.gpsimd.partition_broadcast(out_ap=invB1, in_ap=inv1[0:1, :])
                    nc.vector.tensor_tensor(
                        out=CX[0:64, pr, qsl], in0=c0[0:64, :], in1=invB0,
                        op=mybir.AluOpType.mult,
                    )
                    nc.vector.tensor_tensor(
                        out=CX[64:128, pr, qsl], in0=c1[0:64, :], in1=invB1,
                        op=mybir.AluOpType.mult,
                    )
                # drain any leftover interleave slots, then hand off
                if pending is not None:
                    fn, et = pending
                    for e in range(et, 8):
                        fn(e)
                pending = (emit_outproj(qb), 0)
            # final q block's output projection
            fn, et = pending
            for e in range(et, 8):
                fn(e)

    nc.compile()
    _BUILT = nc
    return nc


def _make_in_maps(inputs) -> list:
    import ml_dtypes

    bf16 = ml_dtypes.bfloat16
    query = np.asarray(inputs["query"], dtype=np.float32)
    key_ = np.asarray(inputs["key_"], dtype=np.float32)
    value = np.asarray(inputs["value"], dtype=np.float32)
    Wq = np.asarray(inputs["Wq"], dtype=np.float32)
    bq = np.asarray(inputs["bq"], dtype=np.float32)
    Wk = np.asarray(inputs["Wk"], dtype=np.float32)
    bk = np.asarray(inputs["bk"], dtype=np.float32)
    Wv = np.asarray(inputs["Wv"], dtype=np.float32)
    bv = np.asarray(inputs["bv"], dtype=np.float32)
    Wo = np.asarray(inputs["Wo"], dtype=np.float32)

    WqT = Wq.T.astype(bf16)  # [E_in, E_out]
    WkT = Wk.T.astype(bf16)
    WvT = Wv.T.astype(bf16)
    WoT = Wo.T.astype(bf16)  # [F_in, E_out]

    xqs = [np.ascontiguousarray(query[b].T).astype(bf16) for b in range(B)]
    xks = [np.ascontiguousarray(key_[b].T).astype(bf16) for b in range(B)]
    xvs = [np.ascontiguousarray(value[b].T).astype(bf16) for b in range(B)]

    in_maps = []
    for c in range(NCORES):
        b = c // 2
        hh = c % 2
        fsl = slice(hh * F, (hh + 1) * F)
        in_maps.append(
            {
                "xq": xqs[b],
                "xk": xks[b],
                "xv": xvs[b],
                "wq": np.ascontiguousarray(WqT[:, fsl]),
                "wk": np.ascontiguousarray(WkT[:, fsl]),
                "wv": np.ascontiguousarray(WvT[:, fsl]),
                "wo": np.ascontiguousarray(WoT[fsl, :]),
                "bq": np.ascontiguousarray(bq[fsl]),
                "bk": np.ascontiguousarray(bk[fsl]),
                "bv": np.ascontiguousarray(bv[fsl]),
            }
        )
    return in_maps


def kernel(**inputs) -> np.ndarray:
    from concourse.bass_utils import run_bass_kernel_spmd

    bo = np.asarray(inputs["bo"], dtype=np.float32)
    nc = _build_program()
    in_maps = _make_in_maps(inputs)

    res = run_bass_kernel_spmd(nc, in_maps, core_ids=list(range(NCORES)))

    out = np.empty((B, S, E), dtype=np.float32)
    for b in range(B):
        partial = res.results[2 * b]["out"] + res.results[2 * b + 1]["out"]  # [E, S]
        out[b] = partial.T + bo[None, :]
    return out
